# revision 1
# baseline (speedup 1.0000x reference)
"""Trainium2 Bass kernel for nn_Block_45552423141629 (pre-norm transformer
block with ELU linear attention), SPMD over 8 NeuronCores.

Sharding: sequence dimension N=8192 split into 8 shards of 1024 tokens; the
kv outer-product statistics ([B,H,64,64] + k_sum) are AllReduce'd across
cores once per batch. Everything else is fully local.

Self-contained: hardcodes shapes from the problem spec.
"""
import contextlib

import numpy as np
import ml_dtypes

import concourse.bass as bass
import concourse.mybir as mybir
import concourse.tile as tile
from concourse import bass_utils
from concourse.vector_clock import ScopedClock

# ---------------------------------------------------------------------------
# Workarounds: this walrus build accepts only ONE sync-wait per instruction.
# Split multi-waits onto unfusable NOPs on the same engine, and do the same
# for the TileContext tail drain.
# ---------------------------------------------------------------------------
_orig_lower = tile.TileContext._lower_ordered_insts


def _split_multi_waits(self, ordered):
    nc = self.nc
    for bb, insts in list(ordered.items()):
        new = []
        changed = False
        for inst in insts:
            si = inst.sync_info
            if si is not None and len(si.on_wait) > 1:
                waits = list(si.on_wait)
                for w in waits[:-1]:
                    nop = mybir.InstNoOp(
                        name=nc.get_next_instruction_name(),
                        ins=[],
                        outs=[],
                        bass_is_fusable=False,
                    )
                    nop.engine = inst.engine
                    nop.sync_info = mybir.SyncInfo(on_wait=[w], on_update=[])
                    new.append(nop)
                inst.sync_info = mybir.SyncInfo(
                    on_wait=[waits[-1]], on_update=list(si.on_update)
                )
                changed = True
            new.append(inst)
        if changed:
            ordered[bb] = new
    return _orig_lower(self, ordered)


if tile.TileContext._lower_ordered_insts is not _split_multi_waits:
    tile.TileContext._lower_ordered_insts = _split_multi_waits


def _patched_drain_and_barrier(self, tick_clock, wait_clock):
    nc = self.nc
    pre = nc.sync.nop(nofuse=True)
    wait_clock.add_sem_waits(pre.ins, ScopedClock({None: tick_clock.global_clock}))
    si = pre.ins.sync_info
    waits = list(si.on_wait) if si is not None else []
    if len(waits) > 1:
        pre.ins.sync_info = mybir.SyncInfo(
            on_wait=[waits[0]], on_update=list(si.on_update)
        )
        for w in waits[1:]:
            n2 = nc.sync.nop(nofuse=True)
            n2.ins.sync_info = mybir.SyncInfo(on_wait=[w], on_update=[])
    nc.sync.drain()
    nc.all_engine_barrier()
    popped = nc._tile_sem_poison_stack.pop()
    assert popped is self._sem_poison
    nc.clear_and_free_semaphores(list(self.sems.allocated().values()))
    nc.all_engine_barrier()


tile.TileContext._drain_and_barrier = _patched_drain_and_barrier

# ---------------------------------------------------------------------------

BF = ml_dtypes.bfloat16
F32 = mybir.dt.float32
BF16 = mybir.dt.bfloat16
AF = mybir.ActivationFunctionType
ALU = mybir.AluOpType

N_CORES = 8
B, N, D, H, HD, DFF = 4, 8192, 1024, 16, 64, 4096
NLOC = N // N_CORES        # 1024 tokens per core per batch
TC = NLOC // 128           # 8 token chunks per batch
DC = D // 128              # 8 dim chunks
GC = DFF // 128            # 32 ff chunks
NPAIR = H // 2             # 8 head pairs
EPS_LN = 1e-5
EPS_NORM = 1e-6

_nc_cache = {}


def _build(has_ckv: bool, has_c2: bool):
    key = (has_ckv, has_c2)
    if key in _nc_cache:
        return _nc_cache[key]

    nc = bass.Bass("TRN2", target_bir_lowering=False, debug=False,
                   num_devices=N_CORES)
    src = nc.dram_tensor("src", [B, NLOC, D], F32, kind="ExternalInput")
    # wq is packed [m, p, j*128+o] (stationary-tile layout, like fc1)
    wq = nc.dram_tensor("wq", [DC, 128, D], BF16, kind="ExternalInput")
    wk = nc.dram_tensor("wk", [DC, 128, D], BF16, kind="ExternalInput")
    wv = nc.dram_tensor("wv", [DC, 128, D], BF16, kind="ExternalInput")
    wo = nc.dram_tensor("wo", [DC, 128, D], BF16, kind="ExternalInput")
    fc1 = nc.dram_tensor("fc1", [GC, 128, D], BF16, kind="ExternalInput")
    fc2 = nc.dram_tensor("fc2", [GC, 128, D], BF16, kind="ExternalInput")
    c1 = nc.dram_tensor("c1", [128, GC], F32, kind="ExternalInput")
    cq = nc.dram_tensor("cq", [128, DC], F32, kind="ExternalInput")
    if has_ckv:
        ckv = nc.dram_tensor("ckv", [2, D], F32, kind="ExternalInput")
    if has_c2:
        c2 = nc.dram_tensor("c2", [D], F32, kind="ExternalInput")
    out = nc.dram_tensor("out", [B, NLOC, D], F32, kind="ExternalOutput")

    with tile.TileContext(nc) as tc:
        ctx = contextlib.ExitStack()
        with ctx:
            p_w = ctx.enter_context(tc.tile_pool(name="p_w", bufs=16))
            p_fc1 = ctx.enter_context(tc.tile_pool(name="p_fc1", bufs=3))
            p_fc2 = ctx.enter_context(tc.tile_pool(name="p_fc2", bufs=3))
            p_x = ctx.enter_context(tc.tile_pool(name="p_x", bufs=2))
            p_hT = ctx.enter_context(tc.tile_pool(name="p_hT", bufs=DC))
            p_qT = ctx.enter_context(tc.tile_pool(name="p_qT", bufs=DC))
            p_k = ctx.enter_context(tc.tile_pool(name="p_k", bufs=3))
            p_v = ctx.enter_context(tc.tile_pool(name="p_v", bufs=3))
            p_aT = ctx.enter_context(tc.tile_pool(name="p_aT", bufs=NPAIR))
            p_s2 = ctx.enter_context(tc.tile_pool(name="p_s2", bufs=1))
            p_h2T = ctx.enter_context(tc.tile_pool(name="p_h2T", bufs=DC))
            p_gt = ctx.enter_context(tc.tile_pool(name="p_gt", bufs=GC))
            p_rnb = ctx.enter_context(tc.tile_pool(name="p_rnb", bufs=2))
            p_tmp = ctx.enter_context(tc.tile_pool(name="p_tmp", bufs=2))
            p_ae = ctx.enter_context(tc.tile_pool(name="p_ae", bufs=4))
            p_sm = ctx.enter_context(tc.tile_pool(name="p_sm", bufs=1))
            p_st = ctx.enter_context(tc.tile_pool(name="p_st", bufs=3))
            p_one = ctx.enter_context(tc.tile_pool(name="p_one", bufs=1))
            p_ob = ctx.enter_context(tc.tile_pool(name="p_ob", bufs=2))
            ps_mm = ctx.enter_context(
                tc.tile_pool(name="ps_mm", bufs=6, space="PSUM"))
            ps_kv = ctx.enter_context(
                tc.tile_pool(name="ps_kv", bufs=1, space="PSUM"))
            dram = ctx.enter_context(
                tc.tile_pool(name="dramp", bufs=4, space="DRAM"))
            dram_s = ctx.enter_context(
                tc.tile_pool(name="dramps", bufs=4, space="DRAM"))
            dram_s2 = ctx.enter_context(
                tc.tile_pool(name="drams2", bufs=2 * TC, space="DRAM"))

            # --- constants ---
            c1_sb = p_one.tile([128, GC], F32, tag="c1", name="c1")
            nc.sync.dma_start(out=c1_sb, in_=c1.ap())
            cq_sb = p_one.tile([128, DC], F32, tag="cq", name="cq")
            nc.sync.dma_start(out=cq_sb, in_=cq.ap())
            eps_sb = p_one.tile([128, 1], F32, tag="eps", name="eps")
            nc.vector.memset(eps_sb, EPS_LN)
            if has_ckv:
                ck_b = p_one.tile([128, D], F32, tag="ckb", name="ckb")
                cv_b = p_one.tile([128, D], F32, tag="cvb", name="cvb")
                ckap = ckv.ap()
                for idx, t in ((0, ck_b), (1, cv_b)):
                    nc.sync.dma_start(
                        out=t,
                        in_=bass.AP(tensor=ckap.tensor, offset=idx * D,
                                    ap=[[0, 128], [1, D]]))
            if has_c2:
                c2_b = p_one.tile([128, D], F32, tag="c2b", name="c2b")
                c2ap = c2.ap()
                nc.sync.dma_start(
                    out=c2_b,
                    in_=bass.AP(tensor=c2ap.tensor, offset=0,
                                ap=[[0, 128], [1, D]]))

            def ln_stats(xt):
                """mean/rstd of [128, D] fp32 tile -> (mv, rstd)."""
                st = p_st.tile([128, 2, 6], F32, tag="st", name="st")
                xr = xt.rearrange("p (s f) -> p s f", s=2)
                for s in range(2):
                    nc.vector.bn_stats(out=st[:, s, :], in_=xr[:, s, :])
                mv = p_st.tile([128, 2], F32, tag="mv", name="mv")
                nc.vector.bn_aggr(out=mv, in_=st)
                rstd = p_st.tile([128, 1], F32, tag="rstd", name="rstd")
                nc.scalar.activation(out=rstd, in_=mv[:, 1:2], func=AF.Sqrt,
                                     bias=eps_sb, scale=1.0)
                nc.vector.reciprocal(out=rstd, in_=rstd)
                return mv, rstd

            for b in range(B):
                # ---------------- Phase A: LN1 + transpose ----------------
                hT = [p_hT.tile([128, NLOC], BF16, tag="hT", name="hT")
                      for _ in range(DC)]
                for i in range(TC):
                    xt = p_x.tile([128, D], F32, tag="x", name="x")
                    nc.sync.dma_start(
                        out=xt, in_=src.ap()[b, i * 128:(i + 1) * 128, :])
                    mv, rstd = ln_stats(xt)
                    h = p_tmp.tile([128, D], BF16, tag="h", name="h")
                    nc.vector.tensor_scalar(
                        out=h, in0=xt, scalar1=mv[:, 0:1], scalar2=rstd,
                        op0=ALU.subtract, op1=ALU.mult)
                    for j in range(DC):
                        nc.sync.dma_start_transpose(
                            hT[j][:, i * 128:(i + 1) * 128],
                            h[:, j * 128:(j + 1) * 128])

                # ------- Phase B: k/v projections + incremental kv --------
                wk_sb = [p_w.tile([128, D], BF16, tag="w", name="wk_sb")
                         for _ in range(DC)]
                wv_sb = [p_w.tile([128, D], BF16, tag="w", name="wv_sb")
                         for _ in range(DC)]
                for j in range(DC):
                    nc.sync.dma_start(out=wk_sb[j], in_=wk.ap()[j])
                for j in range(DC):
                    nc.sync.dma_start(out=wv_sb[j], in_=wv.ap()[j])

                # kv+ksum accumulator: [128, pair, 128] fp32 = 2 PSUM banks;
                # per (pair, head): [64, 65] block at 512B-aligned offsets.
                pkv = ps_kv.tile([128, NPAIR, 128], F32, tag="kv", name="pkv")
                for i in range(TC):
                    k_t = p_k.tile([128, D], BF16, tag="k", name="k_t")
                    v_t = p_v.tile([128, H, HD + 1], BF16, tag="v", name="v_t")
                    nc.vector.memset(v_t[:, :, HD:HD + 1], 1.0)
                    for ncol in range(2):
                        csl = slice(ncol * 512, (ncol + 1) * 512)
                        # k
                        pk = ps_mm.tile([128, 512], F32, tag="mm", name="pk")
                        for j in range(DC):
                            nc.tensor.matmul(
                                pk, hT[j][:, i * 128:(i + 1) * 128],
                                wk_sb[j][:, csl],
                                start=(j == 0), stop=(j == DC - 1))
                        if has_ckv:
                            kb = p_tmp.tile([128, 512], F32, tag="mn",
                                            name="kb")
                            nc.vector.scalar_tensor_tensor(
                                out=kb, in0=pk, scalar=0.0, in1=ck_b[:, csl],
                                op0=ALU.add, op1=ALU.add)
                            ksrc = kb
                        else:
                            ksrc = pk
                        rl = p_ae.tile([128, 512], BF16, tag="ae", name="rl")
                        nc.scalar.activation(out=rl, in_=ksrc, func=AF.Relu)
                        mn = p_tmp.tile([128, 512], F32, tag="mn", name="mn")
                        nc.vector.tensor_scalar_min(out=mn, in0=ksrc,
                                                    scalar1=0.0)
                        ex = p_ae.tile([128, 512], BF16, tag="ae", name="ex")
                        nc.scalar.activation(out=ex, in_=mn, func=AF.Exp)
                        nc.vector.tensor_add(out=k_t[:, csl], in0=ex, in1=rl)
                        # v
                        pv = ps_mm.tile([128, 512], F32, tag="mm", name="pv")
                        for j in range(DC):
                            nc.tensor.matmul(
                                pv, hT[j][:, i * 128:(i + 1) * 128],
                                wv_sb[j][:, csl],
                                start=(j == 0), stop=(j == DC - 1))
                        vdst = v_t[:, ncol * 8:(ncol + 1) * 8, 0:HD]
                        pvr = pv.rearrange("p (h e) -> p h e", e=HD)
                        if has_ckv:
                            cvr = cv_b[:, csl].rearrange(
                                "p (h e) -> p h e", e=HD)
                            nc.vector.scalar_tensor_tensor(
                                out=vdst, in0=pvr, scalar=0.0, in1=cvr,
                                op0=ALU.add, op1=ALU.add)
                        else:
                            nc.vector.tensor_copy(out=vdst, in_=pvr)
                    # accumulate kv for all head pairs from this chunk
                    for hp in range(NPAIR):
                        hA, hB = 2 * hp, 2 * hp + 1
                        nc.tensor.matmul(
                            pkv[0:64, hp, 0:HD + 1],
                            k_t[:, hA * HD:(hA + 1) * HD],
                            v_t[:, hA, :],
                            start=(i == 0), stop=(i == TC - 1),
                            tile_position=(0, 0), skip_group_check=True)
                        nc.tensor.matmul(
                            pkv[64:128, hp, 0:HD + 1],
                            k_t[:, hB * HD:(hB + 1) * HD],
                            v_t[:, hB, :],
                            start=(i == 0), stop=(i == TC - 1),
                            tile_position=(0, 64), skip_group_check=True)

                kv_sb = p_sm.tile([128, NPAIR, HD + 1], F32, tag="kvsb",
                                  name="kvsb")
                nc.vector.tensor_copy(out=kv_sb, in_=pkv[:, :, 0:HD + 1])
                kv_in = dram.tile([128, NPAIR, HD + 1], F32, tag="kvin",
                                  name="kvin")
                kv_out = dram_s.tile([128, NPAIR, HD + 1], F32, tag="kvout",
                                     name="kvout", addr_space="Shared")
                nc.sync.dma_start(out=kv_in, in_=kv_sb)
                nc.gpsimd.collective_compute(
                    "AllReduce", ALU.add,
                    replica_groups=[list(range(N_CORES))],
                    ins=[kv_in.opt()], outs=[kv_out.opt()])

                # ---------------- Phase B3: q projection (overlaps AR) ----
                qT = [p_qT.tile([128, NLOC], BF16, tag="qT", name="qT")
                      for _ in range(DC)]
                for m in range(DC):
                    wqm = p_fc1.tile([128, DC, 128], BF16, tag="f1",
                                     name="wqm")
                    nc.sync.dma_start(
                        out=wqm,
                        in_=wq.ap()[m].rearrange("p (j e) -> p j e", j=DC))
                    for ncol in range(2):
                        csl = slice(ncol * 512, (ncol + 1) * 512)
                        pq = ps_mm.tile([128, 512], F32, tag="mm", name="pq")
                        for j in range(DC):
                            nc.tensor.matmul(
                                pq, wqm[:, j, :], hT[j][:, csl],
                                start=(j == 0), stop=(j == DC - 1))
                        rl = p_ae.tile([128, 512], BF16, tag="ae", name="rlq")
                        nc.scalar.activation(out=rl, in_=pq, func=AF.Relu,
                                             bias=cq_sb[:, m:m + 1], scale=1.0)
                        mn = p_tmp.tile([128, 512], F32, tag="mn", name="mnq")
                        nc.vector.tensor_scalar(
                            out=mn, in0=pq, scalar1=cq_sb[:, m:m + 1],
                            scalar2=0.0, op0=ALU.add, op1=ALU.min)
                        ex = p_ae.tile([128, 512], BF16, tag="ae", name="exq")
                        nc.scalar.activation(out=ex, in_=mn, func=AF.Exp)
                        nc.vector.tensor_add(out=qT[m][:, csl], in0=ex, in1=rl)

                # ---------------- Phase D: attention ---------------------
                kv_red = p_sm.tile([128, NPAIR, HD + 1], F32, tag="kvred",
                                   name="kvred")
                nc.sync.dma_start(out=kv_red, in_=kv_out)
                kvb = p_sm.tile([128, NPAIR, HD + 1], BF16, tag="kvb",
                                name="kvb")
                nc.vector.tensor_copy(out=kvb, in_=kv_red)

                # normalizers: accumulate block-diag ksum matmuls
                pn = [ps_mm.tile([16, 512], F32, tag="mm", name="pn")
                      for _ in range(2)]
                for hp in range(NPAIR):
                    ks16 = p_sm.tile([128, 16], BF16, tag="ks16", name="ks16",
                                     bufs=NPAIR)
                    nc.vector.memset(ks16, 0.0)
                    nc.vector.tensor_copy(
                        out=ks16[0:64, 2 * hp:2 * hp + 1],
                        in_=kvb[0:64, hp, HD:HD + 1])
                    nc.vector.tensor_copy(
                        out=ks16[64:128, 2 * hp + 1:2 * hp + 2],
                        in_=kvb[64:128, hp, HD:HD + 1])
                    for ncol in range(2):
                        nc.tensor.matmul(
                            pn[ncol], ks16,
                            qT[hp][:, ncol * 512:(ncol + 1) * 512],
                            start=(hp == 0), stop=(hp == NPAIR - 1),
                            skip_group_check=True)
                n16 = p_sm.tile([16, NLOC], F32, tag="n16", name="n16")
                for ncol in range(2):
                    nc.vector.tensor_scalar_add(
                        out=n16[:, ncol * 512:(ncol + 1) * 512],
                        in0=pn[ncol], scalar1=EPS_NORM)
                rn16 = p_sm.tile([16, NLOC], BF16, tag="rn16", name="rn16")
                with nc.allow_low_precision(reason="rn broadcast in bf16"):
                    nc.vector.reciprocal(out=rn16, in_=n16)
                rn_d = dram.tile([16, NLOC], BF16, tag="rnd", name="rnd")
                nc.sync.dma_start(out=rn_d, in_=rn16)

                aT = [p_aT.tile([128, NLOC], BF16, tag="aT", name="aT")
                      for _ in range(NPAIR)]
                for hp in range(NPAIR):
                    rnb = p_rnb.tile([128, NLOC], BF16, tag="rnb", name="rnb")
                    rnap = rn_d.opt()
                    for hh in range(2):
                        nc.sync.dma_start(
                            out=rnb[hh * 64:(hh + 1) * 64, :],
                            in_=bass.AP(
                                tensor=rnap.tensor,
                                offset=rnap.offset + (2 * hp + hh) * NLOC,
                                ap=[[0, 64], [1, NLOC]]))
                    for ncol in range(2):
                        csl = slice(ncol * 512, (ncol + 1) * 512)
                        po = ps_mm.tile([128, 512], F32, tag="mm", name="po")
                        nc.tensor.matmul(
                            po[0:64, :], kvb[0:64, hp, 0:HD],
                            qT[hp][0:64, csl],
                            start=True, stop=True, tile_position=(0, 0))
                        nc.tensor.matmul(
                            po[64:128, :], kvb[64:128, hp, 0:HD],
                            qT[hp][64:128, csl],
                            start=True, stop=True, tile_position=(64, 64))
                        nc.vector.tensor_mul(
                            out=aT[hp][:, csl], in0=po, in1=rnb[:, csl])

                # ---------------- Phase E: wo + residual + LN2 -----------
                wo_sb = [p_w.tile([128, D], BF16, tag="w", name="wo_sb")
                         for _ in range(DC)]
                for j in range(DC):
                    nc.sync.dma_start(out=wo_sb[j], in_=wo.ap()[j])
                h2T = [p_h2T.tile([128, NLOC], BF16, tag="h2T", name="h2T")
                       for _ in range(DC)]
                s2d = [dram_s2.tile([128, D], F32, tag="s2d", name="s2d")
                       for _ in range(TC)]
                for i in range(TC):
                    x2 = p_x.tile([128, D], F32, tag="x", name="x2")
                    nc.sync.dma_start(
                        out=x2, in_=src.ap()[b, i * 128:(i + 1) * 128, :])
                    s2 = p_s2.tile([128, D], F32, tag="s2", name="s2")
                    for ncol in range(2):
                        csl = slice(ncol * 512, (ncol + 1) * 512)
                        py = ps_mm.tile([128, 512], F32, tag="mm", name="py")
                        for hp in range(NPAIR):
                            nc.tensor.matmul(
                                py, aT[hp][:, i * 128:(i + 1) * 128],
                                wo_sb[hp][:, csl],
                                start=(hp == 0), stop=(hp == NPAIR - 1))
                        nc.vector.tensor_add(out=s2[:, csl], in0=py,
                                             in1=x2[:, csl])
                    nc.sync.dma_start(out=s2d[i], in_=s2)
                    mv2, rstd2 = ln_stats(s2)
                    h2 = p_tmp.tile([128, D], BF16, tag="h", name="h2")
                    nc.vector.tensor_scalar(
                        out=h2, in0=s2, scalar1=mv2[:, 0:1], scalar2=rstd2,
                        op0=ALU.subtract, op1=ALU.mult)
                    for j in range(DC):
                        nc.sync.dma_start_transpose(
                            h2T[j][:, i * 128:(i + 1) * 128],
                            h2[:, j * 128:(j + 1) * 128])

                # ---------------- Phase G/H: MLP, per t-half -------------
                for half in range(2):
                    tsl = slice(half * 512, (half + 1) * 512)
                    gt = [p_gt.tile([128, 512], BF16, tag="gt", name="gt")
                          for _ in range(GC)]
                    for m in range(GC):
                        f1 = p_fc1.tile([128, DC, 128], BF16, tag="f1",
                                        name="f1")
                        nc.sync.dma_start(
                            out=f1,
                            in_=fc1.ap()[m].rearrange("p (j e) -> p j e",
                                                      j=DC))
                        pu = ps_mm.tile([128, 512], F32, tag="mm", name="pu")
                        for j in range(DC):
                            nc.tensor.matmul(
                                pu, f1[:, j, :], h2T[j][:, tsl],
                                start=(j == 0), stop=(j == DC - 1))
                        nc.scalar.activation(out=gt[m], in_=pu, func=AF.Gelu,
                                             bias=c1_sb[:, m:m + 1], scale=1.0)
                    for ncol in range(2):
                        csl = slice(ncol * 512, (ncol + 1) * 512)
                        py2 = [ps_mm.tile([128, 512], F32, tag="mm",
                                          name="py2") for _ in range(4)]
                        for m in range(GC):
                            f2 = p_fc2.tile([128, 512], BF16, tag="f2",
                                            name="f2")
                            nc.sync.dma_start(out=f2, in_=fc2.ap()[m][:, csl])
                            for ii in range(4):
                                nc.tensor.matmul(
                                    py2[ii],
                                    gt[m][:, ii * 128:(ii + 1) * 128], f2,
                                    start=(m == 0), stop=(m == GC - 1))
                        for ii in range(4):
                            i = half * 4 + ii
                            s2c = p_ob.tile([128, 512], F32, tag="s2c",
                                            name="s2c")
                            nc.sync.dma_start(out=s2c, in_=s2d[i][:, csl])
                            ot = p_ob.tile([128, 512], F32, tag="ot",
                                           name="ot")
                            if has_c2:
                                nc.vector.scalar_tensor_tensor(
                                    out=ot, in0=py2[ii], scalar=0.0,
                                    in1=c2_b[:, csl], op0=ALU.add, op1=ALU.add)
                                nc.vector.tensor_add(out=ot, in0=ot, in1=s2c)
                            else:
                                nc.vector.tensor_add(out=ot, in0=py2[ii],
                                                     in1=s2c)
                            nc.sync.dma_start(
                                out=out.ap()[b, i * 128:(i + 1) * 128, csl],
                                in_=ot)

    _nc_cache[key] = nc
    return nc


def kernel(**inputs) -> np.ndarray:
    src = np.ascontiguousarray(np.asarray(inputs["src"], dtype=np.float32))
    ln1_w = np.asarray(inputs["ln1_w"], np.float32)
    ln1_b = np.asarray(inputs["ln1_b"], np.float32)
    wq = np.asarray(inputs["wq"], np.float32)
    wk = np.asarray(inputs["wk"], np.float32)
    wv = np.asarray(inputs["wv"], np.float32)
    wo = np.asarray(inputs["wo"], np.float32)
    ln2_w = np.asarray(inputs["ln2_w"], np.float32)
    ln2_b = np.asarray(inputs["ln2_b"], np.float32)
    fc1_w = np.asarray(inputs["fc1_w"], np.float32)
    fc1_b = np.asarray(inputs["fc1_b"], np.float32)
    fc2_w = np.asarray(inputs["fc2_w"], np.float32)
    fc2_b = np.asarray(inputs["fc2_b"], np.float32)

    # host-side folds (exact, input-value independent transformations)
    wqf = ((ln1_w[:, None] * wq).astype(BF)
           .reshape(DC, 128, DC, 128).transpose(2, 1, 0, 3)
           .reshape(DC, 128, D).copy())
    wkf = (ln1_w[:, None] * wk).astype(BF).reshape(DC, 128, D)
    wvf = (ln1_w[:, None] * wv).astype(BF).reshape(DC, 128, D)
    wof = wo.astype(BF).reshape(DC, 128, D)
    fc1f = ((ln2_w[:, None] * fc1_w).astype(BF)
            .reshape(DC, 128, GC, 128).transpose(2, 1, 0, 3)
            .reshape(GC, 128, D).copy())
    fc2f = fc2_w.astype(BF).reshape(GC, 128, D)
    cq_v = ln1_b @ wq
    ck_v = ln1_b @ wk
    cv_v = ln1_b @ wv
    c1_v = ln2_b @ fc1_w + fc1_b
    has_ckv = bool(np.any(ck_v) or np.any(cv_v))
    has_c2 = bool(np.any(fc2_b))

    base = {
        "wq": wqf, "wk": wkf, "wv": wvf, "wo": wof,
        "fc1": fc1f, "fc2": fc2f,
        "c1": np.ascontiguousarray(c1_v.reshape(GC, 128).T.astype(np.float32)),
        "cq": np.ascontiguousarray(cq_v.reshape(DC, 128).T.astype(np.float32)),
    }
    if has_ckv:
        base["ckv"] = np.stack([ck_v, cv_v]).astype(np.float32)
    if has_c2:
        base["c2"] = fc2_b.astype(np.float32)

    nc = _build(has_ckv, has_c2)
    in_maps = []
    for c in range(N_CORES):
        m = dict(base)
        m["src"] = np.ascontiguousarray(src[:, c * NLOC:(c + 1) * NLOC, :])
        in_maps.append(m)
    res = bass_utils.run_bass_kernel_spmd(
        nc, in_maps, core_ids=list(range(N_CORES)))
    return np.concatenate(
        [res.results[c]["out"] for c in range(N_CORES)], axis=1)



# revision 5
# speedup vs baseline: 1.0769x; 1.0769x over previous
"""Trainium2 Bass kernel for nn_Block_45552423141629 (pre-norm transformer
block with ELU linear attention), SPMD over 8 NeuronCores.

Sharding: sequence dimension N=8192 split into 8 shards of 1024 tokens; the
kv outer-product statistics ([B,H,64,65] incl. k_sum) are AllReduce'd across
cores once per batch. Everything else is fully local.

v2: software-pipelined across batches so the PE systolic array never idles
(keeps the HAM clock gate warm at 2.4 GHz), weight streams on the scalar
HWDGE queue (no head-of-line blocking behind xbar transposes), s2 residual
kept in SBUF (bf16) instead of a DRAM round-trip, and each batch's kv
AllReduce is kicked off one MLP-phase early so its latency is fully hidden.

Self-contained: hardcodes shapes from the problem spec.
"""
import contextlib

import numpy as np
import ml_dtypes

import concourse.bass as bass
import concourse.mybir as mybir
import concourse.tile as tile
from concourse import bass_utils
from concourse.vector_clock import ScopedClock

# ---------------------------------------------------------------------------
# Workarounds: this walrus build accepts only ONE sync-wait per instruction.
# Split multi-waits onto unfusable NOPs on the same engine, and do the same
# for the TileContext tail drain.
# ---------------------------------------------------------------------------
_orig_lower = tile.TileContext._lower_ordered_insts


def _split_multi_waits(self, ordered):
    nc = self.nc
    for bb, insts in list(ordered.items()):
        new = []
        changed = False
        for inst in insts:
            si = inst.sync_info
            if si is not None and len(si.on_wait) > 1:
                waits = list(si.on_wait)
                for w in waits[:-1]:
                    nop = mybir.InstNoOp(
                        name=nc.get_next_instruction_name(),
                        ins=[],
                        outs=[],
                        bass_is_fusable=False,
                    )
                    nop.engine = inst.engine
                    nop.sync_info = mybir.SyncInfo(on_wait=[w], on_update=[])
                    new.append(nop)
                inst.sync_info = mybir.SyncInfo(
                    on_wait=[waits[-1]], on_update=list(si.on_update)
                )
                changed = True
            new.append(inst)
        if changed:
            ordered[bb] = new
    return _orig_lower(self, ordered)


if tile.TileContext._lower_ordered_insts is not _split_multi_waits:
    tile.TileContext._lower_ordered_insts = _split_multi_waits


def _patched_drain_and_barrier(self, tick_clock, wait_clock):
    nc = self.nc
    pre = nc.sync.nop(nofuse=True)
    wait_clock.add_sem_waits(pre.ins, ScopedClock({None: tick_clock.global_clock}))
    si = pre.ins.sync_info
    waits = list(si.on_wait) if si is not None else []
    if len(waits) > 1:
        pre.ins.sync_info = mybir.SyncInfo(
            on_wait=[waits[0]], on_update=list(si.on_update)
        )
        for w in waits[1:]:
            n2 = nc.sync.nop(nofuse=True)
            n2.ins.sync_info = mybir.SyncInfo(on_wait=[w], on_update=[])
    nc.sync.drain()
    nc.all_engine_barrier()
    popped = nc._tile_sem_poison_stack.pop()
    assert popped is self._sem_poison
    nc.clear_and_free_semaphores(list(self.sems.allocated().values()))
    nc.all_engine_barrier()


tile.TileContext._drain_and_barrier = _patched_drain_and_barrier

# ---------------------------------------------------------------------------

BF = ml_dtypes.bfloat16
F32 = mybir.dt.float32
BF16 = mybir.dt.bfloat16
AF = mybir.ActivationFunctionType
ALU = mybir.AluOpType

N_CORES = 8
B, N, D, H, HD, DFF = 4, 8192, 1024, 16, 64, 4096
NLOC = N // N_CORES        # 1024 tokens per core per batch
TC = NLOC // 128           # 8 token chunks per batch
DC = D // 128              # 8 dim chunks
GC = DFF // 128            # 32 ff chunks
NPAIR = H // 2             # 8 head pairs
EPS_LN = 1e-5
EPS_NORM = 1e-6

_nc_cache = {}


def _build(has_ckv: bool, has_c2: bool):
    key = (has_ckv, has_c2)
    if key in _nc_cache:
        return _nc_cache[key]

    nc = bass.Bass("TRN2", target_bir_lowering=False, debug=False,
                   num_devices=N_CORES)
    src = nc.dram_tensor("src", [B, NLOC, D], F32, kind="ExternalInput")
    # wq is packed [m, p, j*128+o] (stationary-tile layout, like fc1)
    wq = nc.dram_tensor("wq", [DC, 128, D], BF16, kind="ExternalInput")
    wk = nc.dram_tensor("wk", [DC, 128, D], BF16, kind="ExternalInput")
    wv = nc.dram_tensor("wv", [DC, 128, D], BF16, kind="ExternalInput")
    wo = nc.dram_tensor("wo", [DC, 128, D], BF16, kind="ExternalInput")
    fc1 = nc.dram_tensor("fc1", [GC, 128, D], BF16, kind="ExternalInput")
    fc2 = nc.dram_tensor("fc2", [GC, 128, D], BF16, kind="ExternalInput")
    c1 = nc.dram_tensor("c1", [128, GC], F32, kind="ExternalInput")
    cq = nc.dram_tensor("cq", [128, DC], F32, kind="ExternalInput")
    if has_ckv:
        ckv = nc.dram_tensor("ckv", [2, D], F32, kind="ExternalInput")
    if has_c2:
        c2 = nc.dram_tensor("c2", [D], F32, kind="ExternalInput")
    out = nc.dram_tensor("out", [B, NLOC, D], F32, kind="ExternalOutput")

    with tile.TileContext(nc) as tc:
        ctx = contextlib.ExitStack()
        with ctx:
            p_w = ctx.enter_context(tc.tile_pool(name="p_w", bufs=20))
            p_f1 = ctx.enter_context(tc.tile_pool(name="p_f1", bufs=3))
            p_f2 = ctx.enter_context(tc.tile_pool(name="p_f2", bufs=4))
            p_x = ctx.enter_context(tc.tile_pool(name="p_x", bufs=2))
            p_h = ctx.enter_context(tc.tile_pool(name="p_h", bufs=2))
            p_hT = ctx.enter_context(tc.tile_pool(name="p_hT", bufs=DC + 4))
            p_qT = ctx.enter_context(tc.tile_pool(name="p_qT", bufs=DC))
            p_k = ctx.enter_context(tc.tile_pool(name="p_k", bufs=3))
            p_v = ctx.enter_context(tc.tile_pool(name="p_v", bufs=3))
            p_h2T = ctx.enter_context(tc.tile_pool(name="p_h2T", bufs=DC))
            p_s2 = ctx.enter_context(tc.tile_pool(name="p_s2", bufs=TC + 1))
            p_gt = ctx.enter_context(tc.tile_pool(name="p_gt", bufs=GC))
            p_rnb = ctx.enter_context(tc.tile_pool(name="p_rnb", bufs=2))
            p_mn = ctx.enter_context(tc.tile_pool(name="p_mn", bufs=2))
            p_ae = ctx.enter_context(tc.tile_pool(name="p_ae", bufs=4))
            p_sm = ctx.enter_context(tc.tile_pool(name="p_sm", bufs=1))
            p_st = ctx.enter_context(tc.tile_pool(name="p_st", bufs=4))
            p_one = ctx.enter_context(tc.tile_pool(name="p_one", bufs=1))
            p_ob = ctx.enter_context(tc.tile_pool(name="p_ob", bufs=2))
            ps = ctx.enter_context(
                tc.tile_pool(name="ps", bufs=6, space="PSUM"))
            ps_kv = ctx.enter_context(
                tc.tile_pool(name="ps_kv", bufs=1, space="PSUM"))
            dram = ctx.enter_context(
                tc.tile_pool(name="dramp", bufs=4, space="DRAM"))
            dram_s = ctx.enter_context(
                tc.tile_pool(name="dramps", bufs=4, space="DRAM"))

            # --- constants ---
            c1_sb = p_one.tile([128, GC], F32, tag="c1", name="c1")
            nc.sync.dma_start(out=c1_sb, in_=c1.ap())
            cq_sb = p_one.tile([128, DC], F32, tag="cq", name="cq")
            nc.sync.dma_start(out=cq_sb, in_=cq.ap())
            eps_sb = p_one.tile([128, 1], F32, tag="eps", name="eps")
            nc.vector.memset(eps_sb, EPS_LN)
            if has_ckv:
                ck_b = p_one.tile([128, D], F32, tag="ckb", name="ckb")
                cv_b = p_one.tile([128, D], F32, tag="cvb", name="cvb")
                ckap = ckv.ap()
                for idx, t in ((0, ck_b), (1, cv_b)):
                    nc.sync.dma_start(
                        out=t,
                        in_=bass.AP(tensor=ckap.tensor, offset=idx * D,
                                    ap=[[0, 128], [1, D]]))
            if has_c2:
                c2_b = p_one.tile([128, D], F32, tag="c2b", name="c2b")
                c2ap = c2.ap()
                nc.sync.dma_start(
                    out=c2_b,
                    in_=bass.AP(tensor=c2ap.tensor, offset=0,
                                ap=[[0, 128], [1, D]]))

            def ln_stats(xt):
                """mean/rstd of [128, D] tile -> (mv, rstd)."""
                st = p_st.tile([128, 2, 6], F32, tag="st", name="st")
                xr = xt.rearrange("p (s f) -> p s f", s=2)
                for s in range(2):
                    nc.vector.bn_stats(out=st[:, s, :], in_=xr[:, s, :])
                mv = p_st.tile([128, 2], F32, tag="mv", name="mv")
                nc.vector.bn_aggr(out=mv, in_=st)
                rstd = p_st.tile([128, 1], F32, tag="rstd", name="rstd")
                nc.scalar.activation(out=rstd, in_=mv[:, 1:2], func=AF.Sqrt,
                                     bias=eps_sb, scale=1.0)
                nc.vector.reciprocal(out=rstd, in_=rstd)
                return mv, rstd

            # ---------------- Phase A: LN1 + transpose --------------------
            def phase_A(b):
                hT = [p_hT.tile([128, NLOC], BF16, tag="hT", name="hT")
                      for _ in range(DC)]
                for i in range(TC):
                    xt = p_x.tile([128, D], F32, tag="x", name="x")
                    nc.sync.dma_start(
                        out=xt, in_=src.ap()[b, i * 128:(i + 1) * 128, :])
                    mv, rstd = ln_stats(xt)
                    h = p_h.tile([128, D], BF16, tag="h", name="h")
                    nc.vector.tensor_scalar(
                        out=h, in0=xt, scalar1=mv[:, 0:1], scalar2=rstd,
                        op0=ALU.subtract, op1=ALU.mult)
                    for j in range(DC):
                        nc.sync.dma_start_transpose(
                            hT[j][:, i * 128:(i + 1) * 128],
                            h[:, j * 128:(j + 1) * 128])
                return hT

            # ------- Phase B: k/v projections + incremental kv + AR -------
            def phase_B(b, hT):
                wk_sb = [p_w.tile([128, D], BF16, tag="w", name="wk_sb")
                         for _ in range(DC)]
                wv_sb = [p_w.tile([128, D], BF16, tag="w", name="wv_sb")
                         for _ in range(DC)]
                for j in range(DC):
                    nc.scalar.dma_start(out=wk_sb[j], in_=wk.ap()[j])
                for j in range(DC):
                    nc.scalar.dma_start(out=wv_sb[j], in_=wv.ap()[j])

                # kv+ksum accumulator: [128, pair, 128] fp32 = 2 PSUM banks;
                # per (pair, head): [64, 65] block at 512B-aligned offsets.
                pkv = ps_kv.tile([128, NPAIR, 128], F32, tag="kv", name="pkv")
                for i in range(TC):
                    k_t = p_k.tile([128, D], BF16, tag="k", name="k_t")
                    v_t = p_v.tile([128, H, HD + 1], BF16, tag="v", name="v_t")
                    nc.vector.memset(v_t[:, :, HD:HD + 1], 1.0)
                    for ncol in range(2):
                        csl = slice(ncol * 512, (ncol + 1) * 512)
                        # k
                        pk = ps.tile([128, 512], F32, tag="mm", name="pk")
                        for j in range(DC):
                            nc.tensor.matmul(
                                pk, hT[j][:, i * 128:(i + 1) * 128],
                                wk_sb[j][:, csl],
                                start=(j == 0), stop=(j == DC - 1))
                        if has_ckv:
                            kb = p_mn.tile([128, 512], F32, tag="mn",
                                           name="kb")
                            nc.vector.scalar_tensor_tensor(
                                out=kb, in0=pk, scalar=0.0, in1=ck_b[:, csl],
                                op0=ALU.add, op1=ALU.add)
                            ksrc = kb
                        else:
                            ksrc = pk
                        rl = p_ae.tile([128, 512], BF16, tag="ae", name="rl")
                        nc.scalar.activation(out=rl, in_=ksrc, func=AF.Relu)
                        mn = p_mn.tile([128, 512], F32, tag="mn", name="mn")
                        nc.vector.tensor_scalar_min(out=mn, in0=ksrc,
                                                    scalar1=0.0)
                        ex = p_ae.tile([128, 512], BF16, tag="ae", name="ex")
                        nc.scalar.activation(out=ex, in_=mn, func=AF.Exp)
                        nc.vector.tensor_add(out=k_t[:, csl], in0=ex, in1=rl)
                        # v
                        pv = ps.tile([128, 512], F32, tag="mm", name="pv")
                        for j in range(DC):
                            nc.tensor.matmul(
                                pv, hT[j][:, i * 128:(i + 1) * 128],
                                wv_sb[j][:, csl],
                                start=(j == 0), stop=(j == DC - 1))
                        vdst = v_t[:, ncol * 8:(ncol + 1) * 8, 0:HD]
                        pvr = pv.rearrange("p (h e) -> p h e", e=HD)
                        if has_ckv:
                            cvr = cv_b[:, csl].rearrange(
                                "p (h e) -> p h e", e=HD)
                            nc.vector.scalar_tensor_tensor(
                                out=vdst, in0=pvr, scalar=0.0, in1=cvr,
                                op0=ALU.add, op1=ALU.add)
                        else:
                            nc.vector.tensor_copy(out=vdst, in_=pvr)
                    # accumulate kv for all head pairs from this chunk
                    for hp in range(NPAIR):
                        hA, hB = 2 * hp, 2 * hp + 1
                        nc.tensor.matmul(
                            pkv[0:64, hp, 0:HD + 1],
                            k_t[:, hA * HD:(hA + 1) * HD],
                            v_t[:, hA, :],
                            start=(i == 0), stop=(i == TC - 1),
                            tile_position=(0, 0), skip_group_check=True)
                        nc.tensor.matmul(
                            pkv[64:128, hp, 0:HD + 1],
                            k_t[:, hB * HD:(hB + 1) * HD],
                            v_t[:, hB, :],
                            start=(i == 0), stop=(i == TC - 1),
                            tile_position=(0, 64), skip_group_check=True)

                kv_sb = p_sm.tile([128, NPAIR, HD + 1], F32, tag="kvsb",
                                  name="kvsb")
                nc.vector.tensor_copy(out=kv_sb, in_=pkv[:, :, 0:HD + 1])
                kv_in = dram.tile([128, NPAIR, HD + 1], F32, tag="kvin",
                                  name="kvin")
                kv_out = dram_s.tile([128, NPAIR, HD + 1], F32, tag="kvout",
                                     name="kvout", addr_space="Shared")
                nc.scalar.dma_start(out=kv_in, in_=kv_sb)
                nc.gpsimd.collective_compute(
                    "AllReduce", ALU.add,
                    replica_groups=[list(range(N_CORES))],
                    ins=[kv_in.opt()], outs=[kv_out.opt()])
                return kv_out

            # ---------------- Phase B3: q projection ----------------------
            def phase_B3(b, hT):
                qT = [p_qT.tile([128, NLOC], BF16, tag="qT", name="qT")
                      for _ in range(DC)]
                for m in range(DC):
                    wqm = p_f1.tile([128, DC, 128], BF16, tag="f1",
                                    name="wqm")
                    nc.scalar.dma_start(
                        out=wqm,
                        in_=wq.ap()[m].rearrange("p (j e) -> p j e", j=DC))
                    for ncol in range(2):
                        csl = slice(ncol * 512, (ncol + 1) * 512)
                        pq = ps.tile([128, 512], F32, tag="mm", name="pq")
                        for j in range(DC):
                            nc.tensor.matmul(
                                pq, wqm[:, j, :], hT[j][:, csl],
                                start=(j == 0), stop=(j == DC - 1))
                        rl = p_ae.tile([128, 512], BF16, tag="ae", name="rlq")
                        nc.scalar.activation(out=rl, in_=pq, func=AF.Relu,
                                             bias=cq_sb[:, m:m + 1], scale=1.0)
                        mn = p_mn.tile([128, 512], F32, tag="mn", name="mnq")
                        nc.vector.tensor_scalar(
                            out=mn, in0=pq, scalar1=cq_sb[:, m:m + 1],
                            scalar2=0.0, op0=ALU.add, op1=ALU.min)
                        ex = p_ae.tile([128, 512], BF16, tag="ae", name="exq")
                        nc.scalar.activation(out=ex, in_=mn, func=AF.Exp)
                        nc.vector.tensor_add(out=qT[m][:, csl], in0=ex, in1=rl)
                return qT

            # ---------------- Phase D: attention (aT written into qT) -----
            def phase_D(b, qT, kv_out):
                kv_red = p_sm.tile([128, NPAIR, HD + 1], F32, tag="kvred",
                                   name="kvred")
                nc.scalar.dma_start(out=kv_red, in_=kv_out)
                kvb = p_sm.tile([128, NPAIR, HD + 1], BF16, tag="kvb",
                                name="kvb")
                nc.vector.tensor_copy(out=kvb, in_=kv_red)

                # normalizers: accumulate block-diag ksum matmuls
                pn = [ps.tile([16, 512], F32, tag="mm", name="pn")
                      for _ in range(2)]
                for hp in range(NPAIR):
                    ks16 = p_sm.tile([128, 16], BF16, tag="ks16", name="ks16",
                                     bufs=NPAIR)
                    nc.vector.memset(ks16, 0.0)
                    nc.vector.tensor_copy(
                        out=ks16[0:64, 2 * hp:2 * hp + 1],
                        in_=kvb[0:64, hp, HD:HD + 1])
                    nc.vector.tensor_copy(
                        out=ks16[64:128, 2 * hp + 1:2 * hp + 2],
                        in_=kvb[64:128, hp, HD:HD + 1])
                    for ncol in range(2):
                        nc.tensor.matmul(
                            pn[ncol], ks16,
                            qT[hp][:, ncol * 512:(ncol + 1) * 512],
                            start=(hp == 0), stop=(hp == NPAIR - 1),
                            skip_group_check=True)
                # normalizer is O(NLOC) >> EPS_NORM=1e-6; skip the eps add
                rn16 = p_sm.tile([16, NLOC], BF16, tag="rn16", name="rn16")
                with nc.allow_low_precision(reason="rn broadcast in bf16"):
                    for ncol in range(2):
                        nc.vector.reciprocal(
                            out=rn16[:, ncol * 512:(ncol + 1) * 512],
                            in_=pn[ncol])
                rn_d = dram.tile([16, NLOC], BF16, tag="rnd", name="rnd")
                nc.scalar.dma_start(out=rn_d, in_=rn16)

                for hp in range(NPAIR):
                    rnb = p_rnb.tile([128, NLOC], BF16, tag="rnb", name="rnb")
                    rnap = rn_d.opt()
                    for hh in range(2):
                        nc.scalar.dma_start(
                            out=rnb[hh * 64:(hh + 1) * 64, :],
                            in_=bass.AP(
                                tensor=rnap.tensor,
                                offset=rnap.offset + (2 * hp + hh) * NLOC,
                                ap=[[0, 64], [1, NLOC]]))
                    for ncol in range(2):
                        csl = slice(ncol * 512, (ncol + 1) * 512)
                        po = ps.tile([128, 512], F32, tag="mm", name="po")
                        nc.tensor.matmul(
                            po[0:64, :], kvb[0:64, hp, 0:HD],
                            qT[hp][0:64, csl],
                            start=True, stop=True, tile_position=(0, 0))
                        nc.tensor.matmul(
                            po[64:128, :], kvb[64:128, hp, 0:HD],
                            qT[hp][64:128, csl],
                            start=True, stop=True, tile_position=(64, 64))
                        # normalized attention written back into qT (aT)
                        nc.vector.tensor_mul(
                            out=qT[hp][:, csl], in0=po, in1=rnb[:, csl])
                return qT  # now holds aT

            # ---------------- Phase E: wo + residual + LN2 ----------------
            def phase_E(b, aT):
                wo_sb = [p_w.tile([128, D], BF16, tag="w", name="wo_sb")
                         for _ in range(DC)]
                for j in range(DC):
                    nc.scalar.dma_start(out=wo_sb[j], in_=wo.ap()[j])
                h2T = [p_h2T.tile([128, NLOC], BF16, tag="h2T", name="h2T")
                       for _ in range(DC)]
                s2b = [p_s2.tile([128, D], BF16, tag="s2", name="s2b")
                       for _ in range(TC)]
                for i in range(TC):
                    x2 = p_x.tile([128, D], F32, tag="x", name="x2")
                    nc.sync.dma_start(
                        out=x2, in_=src.ap()[b, i * 128:(i + 1) * 128, :])
                    for ncol in range(2):
                        csl = slice(ncol * 512, (ncol + 1) * 512)
                        py = ps.tile([128, 512], F32, tag="mm", name="py")
                        for hp in range(NPAIR):
                            nc.tensor.matmul(
                                py, aT[hp][:, i * 128:(i + 1) * 128],
                                wo_sb[hp][:, csl],
                                start=(hp == 0), stop=(hp == NPAIR - 1))
                        with nc.allow_low_precision(reason="s2 kept bf16"):
                            nc.vector.tensor_add(out=s2b[i][:, csl], in0=py,
                                                 in1=x2[:, csl])
                    mv2, rstd2 = ln_stats(s2b[i])
                    h2 = p_h.tile([128, D], BF16, tag="h", name="h2")
                    nc.vector.tensor_scalar(
                        out=h2, in0=s2b[i], scalar1=mv2[:, 0:1], scalar2=rstd2,
                        op0=ALU.subtract, op1=ALU.mult)
                    for j in range(DC):
                        nc.sync.dma_start_transpose(
                            h2T[j][:, i * 128:(i + 1) * 128],
                            h2[:, j * 128:(j + 1) * 128])
                return h2T, s2b

            # ---------------- Phase G/H: MLP, per t-half ------------------
            def phase_G(b, h2T, s2b):
                for half in range(2):
                    tsl = slice(half * 512, (half + 1) * 512)
                    gt = [p_gt.tile([128, 512], BF16, tag="gt", name="gt")
                          for _ in range(GC)]
                    for m in range(GC):
                        f1 = p_f1.tile([128, DC, 128], BF16, tag="f1",
                                       name="f1")
                        nc.scalar.dma_start(
                            out=f1,
                            in_=fc1.ap()[m].rearrange("p (j e) -> p j e",
                                                      j=DC))
                        pu = ps.tile([128, 512], F32, tag="mm", name="pu")
                        for j in range(DC):
                            nc.tensor.matmul(
                                pu, f1[:, j, :], h2T[j][:, tsl],
                                start=(j == 0), stop=(j == DC - 1))
                        nc.scalar.activation(out=gt[m], in_=pu, func=AF.Gelu,
                                             bias=c1_sb[:, m:m + 1], scale=1.0)
                    for ncol in range(2):
                        csl = slice(ncol * 512, (ncol + 1) * 512)
                        py2 = [ps.tile([128, 512], F32, tag="mm",
                                       name="py2") for _ in range(4)]
                        for m in range(GC):
                            f2 = p_f2.tile([128, 512], BF16, tag="f2",
                                           name="f2")
                            nc.scalar.dma_start(out=f2, in_=fc2.ap()[m][:, csl])
                            for ii in range(4):
                                nc.tensor.matmul(
                                    py2[ii],
                                    gt[m][:, ii * 128:(ii + 1) * 128], f2,
                                    start=(m == 0), stop=(m == GC - 1))
                        for ii in range(4):
                            i = half * 4 + ii
                            ot = p_ob.tile([128, 512], F32, tag="ot",
                                           name="ot")
                            if has_c2:
                                nc.vector.scalar_tensor_tensor(
                                    out=ot, in0=py2[ii], scalar=0.0,
                                    in1=c2_b[:, csl], op0=ALU.add, op1=ALU.add)
                                nc.vector.tensor_add(out=ot, in0=ot,
                                                     in1=s2b[i][:, csl])
                            else:
                                nc.vector.tensor_add(out=ot, in0=py2[ii],
                                                     in1=s2b[i][:, csl])
                            nc.sync.dma_start(
                                out=out.ap()[b, i * 128:(i + 1) * 128, csl],
                                in_=ot)

            # ---------------- software pipeline over batches --------------
            hT = {0: phase_A(0)}
            ar = {0: phase_B(0, hT[0])}
            for b in range(B):
                qT = phase_B3(b, hT[b])
                if b + 1 < B:
                    hT[b + 1] = phase_A(b + 1)
                aT = phase_D(b, qT, ar[b])
                h2T, s2b = phase_E(b, aT)
                if b + 1 < B:
                    ar[b + 1] = phase_B(b + 1, hT[b + 1])
                phase_G(b, h2T, s2b)

    _nc_cache[key] = nc
    return nc


def kernel(**inputs) -> np.ndarray:
    src = np.ascontiguousarray(np.asarray(inputs["src"], dtype=np.float32))
    ln1_w = np.asarray(inputs["ln1_w"], np.float32)
    ln1_b = np.asarray(inputs["ln1_b"], np.float32)
    wq = np.asarray(inputs["wq"], np.float32)
    wk = np.asarray(inputs["wk"], np.float32)
    wv = np.asarray(inputs["wv"], np.float32)
    wo = np.asarray(inputs["wo"], np.float32)
    ln2_w = np.asarray(inputs["ln2_w"], np.float32)
    ln2_b = np.asarray(inputs["ln2_b"], np.float32)
    fc1_w = np.asarray(inputs["fc1_w"], np.float32)
    fc1_b = np.asarray(inputs["fc1_b"], np.float32)
    fc2_w = np.asarray(inputs["fc2_w"], np.float32)
    fc2_b = np.asarray(inputs["fc2_b"], np.float32)

    # host-side folds (exact, input-value independent transformations)
    wqf = ((ln1_w[:, None] * wq).astype(BF)
           .reshape(DC, 128, DC, 128).transpose(2, 1, 0, 3)
           .reshape(DC, 128, D).copy())
    wkf = (ln1_w[:, None] * wk).astype(BF).reshape(DC, 128, D)
    wvf = (ln1_w[:, None] * wv).astype(BF).reshape(DC, 128, D)
    wof = wo.astype(BF).reshape(DC, 128, D)
    fc1f = ((ln2_w[:, None] * fc1_w).astype(BF)
            .reshape(DC, 128, GC, 128).transpose(2, 1, 0, 3)
            .reshape(GC, 128, D).copy())
    fc2f = fc2_w.astype(BF).reshape(GC, 128, D)
    cq_v = ln1_b @ wq
    ck_v = ln1_b @ wk
    cv_v = ln1_b @ wv
    c1_v = ln2_b @ fc1_w + fc1_b
    has_ckv = bool(np.any(ck_v) or np.any(cv_v))
    has_c2 = bool(np.any(fc2_b))

    base = {
        "wq": wqf, "wk": wkf, "wv": wvf, "wo": wof,
        "fc1": fc1f, "fc2": fc2f,
        "c1": np.ascontiguousarray(c1_v.reshape(GC, 128).T.astype(np.float32)),
        "cq": np.ascontiguousarray(cq_v.reshape(DC, 128).T.astype(np.float32)),
    }
    if has_ckv:
        base["ckv"] = np.stack([ck_v, cv_v]).astype(np.float32)
    if has_c2:
        base["c2"] = fc2_b.astype(np.float32)

    nc = _build(has_ckv, has_c2)
    in_maps = []
    for c in range(N_CORES):
        m = dict(base)
        m["src"] = np.ascontiguousarray(src[:, c * NLOC:(c + 1) * NLOC, :])
        in_maps.append(m)
    res = bass_utils.run_bass_kernel_spmd(
        nc, in_maps, core_ids=list(range(N_CORES)))
    return np.concatenate(
        [res.results[c]["out"] for c in range(N_CORES)], axis=1)
